# revision 1
# baseline (speedup 1.0000x reference)
"""NeuroSAT GNN message passing on 8 Trainium2 NeuronCores.

Strategy (graph-data-parallel, 2 graphs per core, zero collectives):
  * The 3-layer MLPs in the reference have no nonlinearity -> each collapses
    to one 64x64 linear, folded on the host into the LSTM input projections.
  * Per-graph scatter-add aggregation over the bipartite clause<->literal
    edges is a dense matmul with the per-graph 440x800 incidence matrix
    (built on host from edge_index). Self-loops fold into the recurrent
    weights; literal-degree bias becomes a host-precomputed [128,800] matrix
    added with one DVE op per gate.
  * Clause LSTM state is only read at clause rows and literal LSTM state at
    literal rows, so each LSTM runs on only its 440/800 rows per graph.
  * flip_perm is a per-graph half-swap of literal columns -> realized by
    reading swapped column ranges.

Layout: the core's 2 graphs share the partition axis: graph0 on partitions
0:64, graph1 on 64:128 of every [128, N] tile (feature-major per half).
Gate matmuls are M=64 per gate, column-tiled (tile_position) so both
graphs' matmuls co-run on the two PE-array column halves. All matmuls are
fp32 (fp32r is tf32 and the recurrence is chaotic: ~5e3x amplification).
"""

import numpy as np

H = 64
ITERS = 24
B, NV, NC, K = 16, 400, 440, 12
NL = 2 * NV                  # literals/graph = 800
NPG = NL + NC                # nodes/graph = 1240
N = B * NPG                  # 19840
NCORES = 8
GPC = B // NCORES            # graphs per core = 2
CHK = 400                    # literal column chunk (aligned to NV flip halves)

_PROGRAM_CACHE = {}


def _build_program():
    from contextlib import ExitStack

    import concourse.bacc as bacc
    import concourse.mybir as mybir
    from concourse.masks import make_identity
    from concourse.tile import TileContext, add_dep_helper

    F32 = mybir.dt.float32
    SIG = mybir.ActivationFunctionType.Sigmoid
    MULT = mybir.AluOpType.mult
    SUB = mybir.AluOpType.subtract

    nc = bacc.Bacc(
        "TRN2", target_bir_lowering=False, debug=False, num_devices=NCORES
    )

    # ---- DRAM I/O (per-core shards; weights replicated) ----
    d_xt_lit = nc.dram_tensor("xt_lit", [3, GPC * NL], F32, kind="ExternalInput")
    d_xt_cl = nc.dram_tensor("xt_cl", [3, GPC * NC], F32, kind="ExternalInput")
    d_at = nc.dram_tensor("at_rm", [GPC, 7, 128, NC], F32, kind="ExternalInput")
    d_a = nc.dram_tensor("a_rm", [GPC, 4, 128, NL], F32, kind="ExternalInput")
    d_wca = nc.dram_tensor("wc_a", [128, 256], F32, kind="ExternalInput")
    d_wcb = nc.dram_tensor("wc_b", [128, 256], F32, kind="ExternalInput")
    d_wc1 = nc.dram_tensor("wc_1", [128, 256], F32, kind="ExternalInput")
    d_wla = nc.dram_tensor("wl_a", [128, 256], F32, kind="ExternalInput")
    d_wlb = nc.dram_tensor("wl_b", [128, 256], F32, kind="ExternalInput")
    d_wlh = nc.dram_tensor("w_lh_dup", [128, 256], F32, kind="ExternalInput")
    d_wcl2 = nc.dram_tensor("w_cl2_dup", [128, 256], F32, kind="ExternalInput")
    d_wv = nc.dram_tensor("wv_dup", [128, 1], F32, kind="ExternalInput")
    d_liw = nc.dram_tensor("li_w3", [3, H], F32, kind="ExternalInput")
    d_ciw = nc.dram_tensor("ci_w3", [3, H], F32, kind="ExternalInput")
    d_bias = nc.dram_tensor("bias_dup", [128, 5], F32, kind="ExternalInput")
    d_dqq = nc.dram_tensor("dqq", [128, 4 * NL], F32, kind="ExternalInput")
    d_out = nc.dram_tensor("vote", [1, GPC * NL], F32, kind="ExternalOutput")

    with TileContext(nc) as tc, ExitStack() as ctx:
        const = ctx.enter_context(tc.tile_pool(name="const", bufs=1))
        state = ctx.enter_context(tc.tile_pool(name="state", bufs=2))
        work = ctx.enter_context(tc.tile_pool(name="work", bufs=2))
        ps = ctx.enter_context(tc.tile_pool(name="ps", bufs=1, space="PSUM"))

        # ---- constants to SBUF ----
        ident = const.tile([128, 128], F32, name="ident")
        make_identity(nc, ident)

        # adjacency chunks are host-padded so every chunk is a full 128 rows
        # (last chunk overlaps the previous one with zeroed overlap rows)
        at_t = const.tile([128, GPC * 7 * NC], F32, name="at_t")  # A^T k-chunks
        for g in range(GPC):
            for k in range(7):
                c0 = NC * (7 * g + k)
                nc.sync.dma_start(out=at_t[:, c0:c0 + NC], in_=d_at[g, k])
        a_t = const.tile([128, GPC * 4 * NL], F32, name="a_t")  # A k-chunks
        for g in range(GPC):
            for k in range(4):
                c0 = NL * (4 * g + k)
                nc.sync.dma_start(out=a_t[:, c0:c0 + NL], in_=d_a[g, k])

        def load(dram, shape, nm):
            t = const.tile(shape, F32, name=nm)
            nc.sync.dma_start(out=t[:, :], in_=dram[:, :])
            return t

        xt_lit = load(d_xt_lit, [3, GPC * NL], "xt_lit_sb")
        xt_cl = load(d_xt_cl, [3, GPC * NC], "xt_cl_sb")
        wc_a = load(d_wca, [128, 256], "wc_a_sb")
        wc_b = load(d_wcb, [128, 256], "wc_b_sb")
        wc_1 = load(d_wc1, [128, 256], "wc_1_sb")
        wl_a = load(d_wla, [128, 256], "wl_a_sb")
        wl_b = load(d_wlb, [128, 256], "wl_b_sb")
        w_lh = load(d_wlh, [128, 256], "w_lh_sb")
        w_cl2 = load(d_wcl2, [128, 256], "w_cl2_sb")
        wv = load(d_wv, [128, 1], "wv_sb")
        li_w = load(d_liw, [3, H], "li_w_sb")
        ci_w = load(d_ciw, [3, H], "ci_w_sb")
        bias = load(d_bias, [128, 5], "bias_sb")
        dqq = load(d_dqq, [128, 4 * NL], "dqq_sb")

        def MM(*a, **kw):
            kw.setdefault("skip_group_check", True)
            return nc.tensor.matmul(*a, **kw)

        TPOS = ((0, 0), (0, 64))  # col-group per graph-half
        LO, HI = slice(0, 64), slice(64, 128)
        HALF = (LO, HI)

        # ---- initial node states (bias via ones row of xt) ----
        lit_h = state.tile([128, NL], F32, tag="lit_h", name="lit_h0")
        for hf in range(2):
            p = ps.tile([128, CHK], F32, tag="g", bufs=4, name=f"ini_{hf}")
            for g in range(GPC):
                MM(p[HALF[g], :], li_w[0:3, :],
                   xt_lit[0:3, g * NL + hf * CHK:g * NL + (hf + 1) * CHK],
                   start=True, stop=True, tile_position=TPOS[g])
            nc.scalar.copy(lit_h[:, hf * CHK:(hf + 1) * CHK], p[:, :])
        cl_h = state.tile([128, NC], F32, tag="cl_h", name="cl_h0")
        pc = ps.tile([128, NC], F32, tag="g", bufs=4, name="ini_c")
        for g in range(GPC):
            MM(pc[HALF[g], :], ci_w[0:3, :], xt_cl[0:3, g * NC:(g + 1) * NC],
               start=True, stop=True, tile_position=TPOS[g])
        nc.scalar.copy(cl_h[:, :], pc[:, :])

        out_lit = lit_h      # [128, 800]: rows 0:64 g0 features, 64:128 g1
        out_cl = cl_h        # [128, 440]
        lit_c = None
        cl_c = None

        for t in range(1, ITERS):
            first = t == 1

            # -- transpose out_lit -> row-major [lit, feat] chunks per graph --
            rm_l = []
            for g in range(GPC):
                tp = ps.tile([128, 7 * H], F32, tag="ta", bufs=2,
                             name=f"tpl_{t}_{g}")
                for k in range(7):
                    c0 = 128 * k if k < 6 else NL - 128
                    nc.tensor.transpose(
                        tp[:, k * H:(k + 1) * H],
                        out_lit[HALF[g], c0:c0 + 128],
                        ident[HALF[g], HALF[g]],
                    )
                rm = work.tile([128, 7 * H], F32, tag="rml", bufs=3, name=f"rml_{t}_{g}")
                nc.scalar.copy(rm[:, :], tp[:, :])
                rm_l.append(rm)

            # -- clause agg: raw A @ out_lit per graph (g0 -> hi, g1 -> lo) --
            agc = ps.tile([128, NC], F32, tag="ta", bufs=2, name=f"agc_{t}")
            prev = None
            for g in range(GPC):
                half = HI if g == 0 else LO
                for k in range(7):
                    mm = MM(agc[half, :], rm_l[g][:, k * H:(k + 1) * H],
                            at_t[:, NC * (7 * g + k):NC * (7 * g + k + 1)],
                            start=(k == 0), stop=(k == 6),
                            tile_position=TPOS[1 - g])
                    if k == 0 and prev is not None:
                        add_dep_helper(mm.ins, prev.ins, sync=True,
                                       reason="psum half-group order")
                    if k == 6:
                        prev = mm
            # stacks: g0 = (ch | agg), g1 = (agg | ch)
            st0 = work.tile([128, NC], F32, tag="stc0", bufs=3, name=f"stc0_{t}")
            st1 = work.tile([128, NC], F32, tag="stc1", bufs=3, name=f"stc1_{t}")
            nc.gpsimd.tensor_copy(st0[LO, :], out_cl[LO, :])
            nc.scalar.copy(st0[HI, :], agc[HI, :])
            nc.scalar.copy(st1[LO, :], agc[LO, :])
            nc.gpsimd.tensor_copy(st1[HI, :], out_cl[HI, :])

            # -- clause gates: 4 gate groups, col-tiled graph pairs --
            wA, wB = (wc_1, wc_1) if first else (wc_a, wc_b)
            gps_c = []
            for x in range(4):
                gp = ps.tile([128, NC], F32, tag="cg", bufs=2, name=f"cg{x}_{t}")
                xs = slice(x * H, (x + 1) * H)
                MM(gp[LO, :], wA[:, xs], st0[:, :], start=True, stop=True,
                   tile_position=(0, 0))
                MM(gp[HI, :], wB[:, xs], st1[:, :], start=True, stop=True,
                   tile_position=(0, 64))
                gps_c.append(gp)
            ch_new = state.tile([128, NC], F32, tag="cl_h", name=f"ch_{t}")
            cc_new = state.tile([128, NC], F32, tag="cl_c", name=f"cc_{t}")
            # tanh(x) = 2*sigmoid(2x) - 1: keeps ACT on one table (no
            # 1283ns table reload between Sigmoid and Tanh)
            s_i = work.tile([128, NC], F32, tag="si", name=f"csi_{t}")
            nc.scalar.activation(s_i[:, :], gps_c[0][:, :], SIG, bias=bias[:, 0:1])
            s_f = work.tile([128, NC], F32, tag="sf", name=f"csf_{t}")
            nc.scalar.activation(s_f[:, :], gps_c[1][:, :], SIG, bias=bias[:, 1:2])
            s_g = work.tile([128, NC], F32, tag="sg", name=f"csg_{t}")
            nc.scalar.activation(s_g[:, :], gps_c[2][:, :], SIG,
                                 bias=bias[:, 2:3], scale=2.0)
            s_o = work.tile([128, NC], F32, tag="so", name=f"cso_{t}")
            nc.scalar.activation(s_o[:, :], gps_c[3][:, :], SIG, bias=bias[:, 3:4])
            t1 = work.tile([128, NC], F32, tag="t1", name=f"ct1_{t}")
            nc.vector.tensor_mul(t1[:, :], s_i[:, :], s_g[:, :])
            if first:
                nc.vector.scalar_tensor_tensor(
                    cc_new[:, :], t1[:, :], 2.0, s_i[:, :],
                    op0=MULT, op1=SUB)
            else:
                u = work.tile([128, NC], F32, tag="u", name=f"cu_{t}")
                nc.vector.scalar_tensor_tensor(
                    u[:, :], t1[:, :], 2.0, s_i[:, :], op0=MULT, op1=SUB)
                t2 = work.tile([128, NC], F32, tag="t2", name=f"ct2_{t}")
                nc.vector.tensor_mul(t2[:, :], s_f[:, :], cl_c[:, :])
                nc.vector.tensor_add(cc_new[:, :], u[:, :], t2[:, :])
            tnc = work.tile([128, NC], F32, tag="tnc", name=f"ctn_{t}")
            nc.scalar.activation(tnc[:, :], cc_new[:, :], SIG, scale=2.0)
            t3 = work.tile([128, NC], F32, tag="t3", name=f"ct3_{t}")
            nc.vector.tensor_mul(t3[:, :], s_o[:, :], tnc[:, :])
            nc.vector.scalar_tensor_tensor(
                ch_new[:, :], t3[:, :], 2.0, s_o[:, :], op0=MULT, op1=SUB)

            # -- transpose ch_new; literal agg per half-chunk --
            rm_c = []
            for g in range(GPC):
                tp = ps.tile([128, 4 * H], F32, tag="ta", bufs=2,
                             name=f"tpc_{t}_{g}")
                for k in range(4):
                    c0 = 128 * k if k < 3 else NC - 128
                    nc.tensor.transpose(
                        tp[:, k * H:(k + 1) * H],
                        ch_new[HALF[g], c0:c0 + 128],
                        ident[HALF[g], HALF[g]],
                    )
                rm = work.tile([128, 4 * H], F32, tag="rmc", bufs=3, name=f"rmc_{t}_{g}")
                nc.scalar.copy(rm[:, :], tp[:, :])
                rm_c.append(rm)

            lh_new = state.tile([128, NL], F32, tag="lit_h", name=f"lh_{t}")
            lc_new = state.tile([128, NL], F32, tag="lit_c", name=f"lc_{t}")
            wS = w_cl2 if first else w_lh
            for hf in range(2):
                cs = slice(hf * CHK, (hf + 1) * CHK)
                fs = slice((1 - hf) * CHK, (2 - hf) * CHK)
                agl = ps.tile([128, CHK], F32, tag="ta", bufs=2,
                              name=f"agl_{t}_{hf}")
                prev = None
                for g in range(GPC):
                    half = HI if g == 0 else LO
                    for k in range(4):
                        mm = MM(agl[half, :], rm_c[g][:, k * H:(k + 1) * H],
                                a_t[:, NL * (4 * g + k) + hf * CHK:
                                    NL * (4 * g + k) + (hf + 1) * CHK],
                                start=(k == 0), stop=(k == 3),
                                tile_position=TPOS[1 - g])
                        if k == 0 and prev is not None:
                            add_dep_helper(mm.ins, prev.ins, sync=True,
                                           reason="psum half-group order")
                        if k == 3:
                            prev = mm
                # stacks: g0 = (flip | agg), g1 = (agg | flip)
                s0 = work.tile([128, CHK], F32, tag="stl0", bufs=3, name=f"sl0_{t}_{hf}")
                s1 = work.tile([128, CHK], F32, tag="stl1", bufs=3, name=f"sl1_{t}_{hf}")
                nc.gpsimd.tensor_copy(s0[LO, :], out_lit[LO, fs])
                nc.scalar.copy(s0[HI, :], agl[HI, :])
                nc.scalar.copy(s1[LO, :], agl[LO, :])
                nc.gpsimd.tensor_copy(s1[HI, :], out_lit[HI, fs])

                gps = []
                for x in range(4):
                    gp = ps.tile([128, CHK], F32, tag="g", bufs=4,
                                 name=f"lg{x}_{t}_{hf}")
                    xs = slice(x * H, (x + 1) * H)
                    MM(gp[LO, :], wS[0:64, xs], out_lit[LO, cs],
                       start=True, stop=False, tile_position=(0, 0))
                    lo2 = MM(gp[LO, :], wl_a[:, xs], s0[:, :], start=False,
                             stop=True, tile_position=(0, 0))
                    hi1 = MM(gp[HI, :], wS[64:128, xs], out_lit[HI, cs],
                             start=True, stop=False, tile_position=(64, 64))
                    add_dep_helper(hi1.ins, lo2.ins, sync=True,
                                   reason="psum half-group order")
                    MM(gp[HI, :], wl_b[:, xs], s1[:, :], start=False,
                       stop=True, tile_position=(0, 64))
                    # degree-dependent bias + lu biases (host-precomputed);
                    # lands in SBUF so the psum bank frees early
                    stg = work.tile([128, CHK], F32, tag=f"stg{x}",
                                    name=f"stg{x}_{t}_{hf}")
                    nc.vector.tensor_add(stg[:, :], gp[:, :],
                                         dqq[:, x * NL + hf * CHK:
                                             x * NL + (hf + 1) * CHK])
                    gps.append(stg)
                s_i = work.tile([128, CHK], F32, tag="si", name=f"lsi_{t}_{hf}")
                nc.scalar.activation(s_i[:, :], gps[0][:, :], SIG)
                s_f = work.tile([128, CHK], F32, tag="sf", name=f"lsf_{t}_{hf}")
                nc.scalar.activation(s_f[:, :], gps[1][:, :], SIG)
                s_g = work.tile([128, CHK], F32, tag="sg", name=f"lsg_{t}_{hf}")
                nc.scalar.activation(s_g[:, :], gps[2][:, :], SIG, scale=2.0)
                s_o = work.tile([128, CHK], F32, tag="so", name=f"lso_{t}_{hf}")
                nc.scalar.activation(s_o[:, :], gps[3][:, :], SIG)
                t1 = work.tile([128, CHK], F32, tag="t1", name=f"lt1_{t}_{hf}")
                nc.vector.tensor_mul(t1[:, :], s_i[:, :], s_g[:, :])
                if first:
                    nc.vector.scalar_tensor_tensor(
                        lc_new[:, cs], t1[:, :], 2.0, s_i[:, :],
                        op0=MULT, op1=SUB)
                else:
                    u = work.tile([128, CHK], F32, tag="u", name=f"lu_{t}_{hf}")
                    nc.vector.scalar_tensor_tensor(
                        u[:, :], t1[:, :], 2.0, s_i[:, :], op0=MULT, op1=SUB)
                    t2 = work.tile([128, CHK], F32, tag="t2", name=f"lt2_{t}_{hf}")
                    nc.vector.tensor_mul(t2[:, :], s_f[:, :], lit_c[:, cs])
                    nc.vector.tensor_add(lc_new[:, cs], u[:, :], t2[:, :])
                tnc = work.tile([128, CHK], F32, tag="tnc", name=f"ltn_{t}_{hf}")
                nc.scalar.activation(tnc[:, :], lc_new[:, cs], SIG, scale=2.0)
                t3 = work.tile([128, CHK], F32, tag="t3", name=f"lt3_{t}_{hf}")
                nc.vector.tensor_mul(t3[:, :], s_o[:, :], tnc[:, :])
                nc.vector.scalar_tensor_tensor(
                    lh_new[:, cs], t3[:, :], 2.0, s_o[:, :], op0=MULT, op1=SUB)

            out_lit, out_cl = lh_new, ch_new
            lit_c, cl_c = lc_new, cc_new

        # ---- vote head ----
        vote_sb = work.tile([1, GPC * NL], F32, tag="vote", name="vote_sb")
        for g in range(GPC):
            for hf in range(2):
                p = ps.tile([1, CHK], F32, tag="ta", bufs=2,
                            name=f"vps_{g}_{hf}")
                MM(p[:, :], wv[HALF[g], 0:1],
                   out_lit[HALF[g], hf * CHK:(hf + 1) * CHK],
                   start=True, stop=True,
                   tile_position=(64 * g, 0))
                nc.scalar.activation(
                    vote_sb[0:1, g * NL + hf * CHK:g * NL + (hf + 1) * CHK],
                    p[:, :], mybir.ActivationFunctionType.Identity,
                    bias=bias[0:1, 4:5],
                )
        nc.sync.dma_start(out=d_out[:, :], in_=vote_sb[:, :])

    nc.compile()
    return nc


def _fold_and_shard(inputs):
    """Host-side preprocessing: fold weights, build adjacency, shard by graph."""
    f32 = np.float32
    g = {k: np.asarray(v) for k, v in inputs.items()}

    def collapse(w1, b1, w2, b2, w3, b3):
        return w1 @ w2 @ w3, ((b1 @ w2) + b2) @ w3 + b3

    Wl, bl = collapse(g["lm1_w"], g["lm1_b"], g["lm2_w"], g["lm2_b"],
                      g["lm3_w"], g["lm3_b"])
    Wc, bc = collapse(g["cm1_w"], g["cm1_b"], g["cm2_w"], g["cm2_b"],
                      g["cm3_w"], g["cm3_b"])
    Wv, bv = collapse(g["lv1_w"], g["lv1_b"], g["lv2_w"], g["lv2_b"],
                      g["lv3_w"], g["lv3_b"])

    cu_wih, lu_wih = g["cu_wih"], g["lu_wih"]
    w_lc = (Wl @ cu_wih).astype(f32)                 # agg_c -> clause gates
    w_ch = (w_lc + g["cu_whh"]).astype(f32)          # t>=2 merged recurrent
    cbias_c = ((K + 1) * (bl @ cu_wih) + g["cu_bih"] + g["cu_bhh"]).astype(f32)
    wih_a = lu_wih[0:H].astype(f32)                  # flip -> lit gates
    w_cl2 = (Wc @ lu_wih[H:2 * H]).astype(f32)       # agg_l -> lit gates
    w_lh = (w_cl2 + g["lu_whh"]).astype(f32)         # t>=2 merged recurrent
    q_l = (bc @ lu_wih[H:2 * H]).astype(f32)         # [256]
    cbias_l = (g["lu_bih"] + g["lu_bhh"]).astype(f32)

    vs = np.vstack
    wc_a = vs([w_ch, w_lc])
    wc_b = vs([w_lc, w_ch])
    wc_1 = vs([w_lc, w_lc])
    wl_a = vs([wih_a, w_cl2])
    wl_b = vs([w_cl2, wih_a])
    w_lh_dup = vs([w_lh, w_lh])
    w_cl2_dup = vs([w_cl2, w_cl2])
    wv_dup = vs([Wv.astype(f32), Wv.astype(f32)])

    bias_dup = np.zeros((128, 5), f32)
    for x in range(4):
        scl = 2.0 if x == 2 else 1.0   # g-gate runs as sigmoid(2x+2b)
        bias_dup[0:64, x] = scl * cbias_c[x * H:(x + 1) * H]
        bias_dup[64:128, x] = scl * cbias_c[x * H:(x + 1) * H]
    bias_dup[0, 4] = bv[0]

    li_w3 = np.concatenate([g["li_w"], g["li_b"][None, :]], axis=0).astype(f32)
    ci_w3 = np.concatenate([g["ci_w"], g["ci_b"][None, :]], axis=0).astype(f32)

    # adjacency per graph from edge_index (direction-robust)
    ei = g["edge_index"].astype(np.int64)
    src, dst = ei[0], ei[1]
    src_g, dst_g = src // NPG, dst // NPG
    assert np.all(src_g == dst_g), "edges must be graph-local"
    src_l, dst_l = src % NPG, dst % NPG
    s_lit, d_lit = src_l < NL, dst_l < NL
    A_in_c = np.zeros((B, NC, NL), f32)   # clause <- literal edges
    m = (~d_lit) & s_lit
    np.add.at(A_in_c, (dst_g[m], dst_l[m] - NL, src_l[m]), 1.0)
    A_in_l = np.zeros((B, NL, NC), f32)   # literal <- clause edges
    m = d_lit & (~s_lit)
    np.add.at(A_in_l, (dst_g[m], dst_l[m], src_l[m] - NL), 1.0)
    deg_l = A_in_l.sum(axis=2)            # [B, NL]

    x = g["x"].astype(f32).reshape(B, NPG, 2)
    ones = np.ones((B, NPG, 1), f32)
    x3 = np.concatenate([x, ones], axis=2)        # [B, NPG, 3]

    shared = dict(
        wc_a=wc_a, wc_b=wc_b, wc_1=wc_1, wl_a=wl_a, wl_b=wl_b,
        w_lh_dup=w_lh_dup, w_cl2_dup=w_cl2_dup, wv_dup=wv_dup,
        li_w3=li_w3, ci_w3=ci_w3, bias_dup=bias_dup,
    )
    in_maps = []
    for c in range(NCORES):
        gs = slice(c * GPC, (c + 1) * GPC)
        x3c = x3[gs]                               # [GPC, NPG, 3]
        xt_lit = np.ascontiguousarray(
            x3c[:, :NL].transpose(2, 0, 1).reshape(3, GPC * NL))
        xt_cl = np.ascontiguousarray(
            x3c[:, NL:].transpose(2, 0, 1).reshape(3, GPC * NC))
        # dqq[x]: rows 0:64 = q_x (x) (deg_g0+1) + cbias_l_x ; rows 64:128 g1
        dqq = np.zeros((128, 4 * NL), f32)
        for x_ in range(4):
            qx = q_l[x_ * H:(x_ + 1) * H]
            cbx = cbias_l[x_ * H:(x_ + 1) * H]
            for gg in range(GPC):
                d1 = deg_l[c * GPC + gg] + 1.0
                dqq[gg * 64:(gg + 1) * 64, x_ * NL:(x_ + 1) * NL] = (
                    np.outer(qx, d1) + cbx[:, None])
        # pre-chunk adjacency into full-128-row K-chunks; the final chunk
        # overlaps the previous one with its overlap rows zeroed
        atc = np.zeros((GPC, 7, 128, NC), f32)
        ac = np.zeros((GPC, 4, 128, NL), f32)
        for gg in range(GPC):
            at_full = A_in_c[c * GPC + gg].T       # [NL, NC]
            a_full = A_in_l[c * GPC + gg].T        # [NC, NL]
            for k in range(6):
                atc[gg, k] = at_full[128 * k:128 * (k + 1)]
            atc[gg, 6, 128 - (NL - 768):] = at_full[768:]
            for k in range(3):
                ac[gg, k] = a_full[128 * k:128 * (k + 1)]
            ac[gg, 3, 128 - (NC - 384):] = a_full[384:]
        in_maps.append(dict(
            xt_lit=xt_lit, xt_cl=xt_cl, at_rm=atc, a_rm=ac,
            dqq=dqq, **shared,
        ))
    return in_maps


_LAST_RESULTS = {}


def kernel(**inputs):
    from concourse.bass_utils import run_bass_kernel_spmd

    in_maps = _fold_and_shard(inputs)
    if "nc" not in _PROGRAM_CACHE:
        _PROGRAM_CACHE["nc"] = _build_program()
    nc = _PROGRAM_CACHE["nc"]
    res = run_bass_kernel_spmd(nc, in_maps, core_ids=list(range(NCORES)))
    _LAST_RESULTS["res"] = res
    out = np.zeros((N, 1), np.float32)
    for c in range(NCORES):
        vote = res.results[c]["vote"].reshape(GPC, NL)
        for g in range(GPC):
            base = (c * GPC + g) * NPG
            out[base:base + NL, 0] = vote[g]
    return out

